# revision 30
# baseline (speedup 1.0000x reference)
"""Distributed Trainium2 kernel for RoPE multi-head attention.

Reference computation (B=2, S=2048, D=1024, H=16, E=64, fp32):
    q = rope(x @ wq); k = rope(x @ wk); v = x @ wv
    o = softmax(q k^T / sqrt(E)) v ; out = o @ wo

Sharding over 8 NeuronCores: core c -> (batch b = c // 4, head group
hg = c % 4 of 4 heads).  Each core computes its heads' attention and a
partial output projection; the host sums the 4 partials per batch
(tensor-parallel unshard).

RoPE is folded into a second projection: rope(q) = (x@wq) * cos_t +
(x@wq2) * sin_t where wq2 has the E-halves swapped/negated (host prep),
so on-device RoPE is 3 elementwise ops against host-built tables.
"""

import sys

for _p in ("/opt/trn_rl_repo", "/opt/pypackages"):
    if _p not in sys.path:
        sys.path.append(_p)

import numpy as np
import ml_dtypes

import concourse.bacc as bacc
import concourse.mybir as mybir
import concourse.tile as tile
from concourse.bass_utils import run_bass_kernel_spmd

F32 = mybir.dt.float32
BF16 = mybir.dt.bfloat16

B, S, D, H, E = 2, 2048, 1024, 16, 64
HL = 4              # heads per core
EL = HL * E         # 256 local e width
N_CORES = 8
MAX_TIMESCALE = 10000.0
MIN_TIMESCALE = 1.0

_CACHE = {}


def _build_graph():
    nc = bacc.Bacc("TRN2", target_bir_lowering=False, debug=False,
                   num_devices=N_CORES)

    x_ext = nc.dram_tensor("x", [S, D], F32, kind="ExternalInput")
    w_ext = {
        name: nc.dram_tensor(name, [D, EL], BF16, kind="ExternalInput")
        for name in ("wq", "wq2", "wk", "wk2", "wv")
    }
    wo_ext = nc.dram_tensor("wo", [EL, D], BF16, kind="ExternalInput")
    cos_ext = nc.dram_tensor("cos_t", [128, S], F32, kind="ExternalInput")
    sin_ext = nc.dram_tensor("sin_t", [128, S], F32, kind="ExternalInput")
    id_ext = nc.dram_tensor("ident", [128, 128], BF16, kind="ExternalInput")
    out_ext = nc.dram_tensor("out", [S, D], F32, kind="ExternalOutput")

    ST = S // 128       # 16 s-tiles
    SBL = S // 512      # 4 s-blocks
    DC = D // 128       # 8 d-chunks

    with tile.TileContext(nc) as tc:
        with (
            tc.tile_pool(name="persist", bufs=1) as pp,
            tc.tile_pool(name="xin", bufs=3) as xin_pool,
            tc.tile_pool(name="work", bufs=4) as wk_pool,
            tc.tile_pool(name="ps", bufs=2, space="PSUM") as ps_pool,
            tc.tile_pool(name="psm", bufs=2, space="PSUM") as psm_pool,
        ):
            LAG = 3

            # ---------- constants / weights ----------
            ident = pp.tile([128, 128], BF16, tag="ident", name="ident")
            nc.sync.dma_start(ident[:], id_ext.ap())
            w_sb = {}

            def load_w(name):
                tiles = []
                for dc in range(DC):
                    t = pp.tile([128, EL], BF16, tag=f"{name}{dc}", name=f"{name}{dc}")
                    nc.sync.dma_start(t[:], w_ext[name].ap()[dc * 128:(dc + 1) * 128, :])
                    tiles.append(t)
                w_sb[name] = tiles


            # ---------- persistent tiles ----------
            # xT split into 4 s-quarters; xT_q[q][:, dc*512+s] = x[q*512+s, dc*128+p]
            xT_q = [pp.tile([128, DC * 512], BF16, tag=f"xTq{q}", name=f"xTq{q}")
                    for q in range(4)]

            def xT(dc, sb):
                return xT_q[sb][:, dc * 512:(dc + 1) * 512]

            def xT128(dc, st):
                q, r = st // 4, st % 4
                return xT_q[q][:, dc * 512 + r * 128:dc * 512 + (r + 1) * 128]

            qT = [pp.tile([128, S], BF16, tag=f"qT{eb}", name=f"qT{eb}") for eb in range(2)]
            kT = [pp.tile([128, S], BF16, tag=f"kT{eb}", name=f"kT{eb}") for eb in range(2)]
            v_sb = [pp.tile([128, HL * 65], BF16, tag=f"v{st}", name=f"v{st}")
                    for st in range(ST)]

            # ---------- emitters ----------
            def emit_x_dma(st):
                x_t = xin_pool.tile([128, D], F32, tag="x_in", name="x_in",
                                    bufs=8)
                nc.sync.dma_start(x_t[:], x_ext.ap()[st * 128:(st + 1) * 128, :])
                return x_t

            def emit_x_tile(st, x_t):
                """cast bf16, PE-transpose into xT_q"""
                xb = xin_pool.tile([128, D], BF16, tag="x_bf", name="x_bf")
                nc.scalar.copy(xb[:], x_t[:])
                ps_t = ps_pool.tile([128, D], BF16, tag="psS", name="tp")
                for dc in range(DC):
                    nc.tensor.matmul(ps_t[:, dc * 128:(dc + 1) * 128],
                                     xb[:, dc * 128:(dc + 1) * 128], ident[:],
                                     is_transpose=True, start=(dc == 0),
                                     stop=(dc == DC - 1))
                q, r = st // 4, st % 4
                dst = xT_q[q][:].rearrange("p (d s) -> p d s", s=512)[:, :, r * 128:(r + 1) * 128]
                src_ap = ps_t[:].rearrange("p (d c) -> p d c", c=128)
                nc.vector.tensor_copy(dst, src_ap)

            def emit_proj(w1n, w2n, dst, eb, sb):
                """rope-folded projection of one [128, 512] block of qT/kT"""
                pj = psm_pool.tile([128, 1024], F32, tag="psM", name="pj")
                psA, psB = pj[:, 0:512], pj[:, 512:1024]
                for dc in range(DC):
                    nc.tensor.matmul(
                        psA, w_sb[w1n][dc][:, eb * 128:(eb + 1) * 128],
                        xT(dc, sb), start=(dc == 0), stop=(dc == DC - 1))
                for dc in range(DC):
                    nc.tensor.matmul(
                        psB, w_sb[w2n][dc][:, eb * 128:(eb + 1) * 128],
                        xT(dc, sb), start=(dc == 0), stop=(dc == DC - 1))
                t1 = wk_pool.tile([128, 512], F32, tag="ropeA", name="ropeA", bufs=6)
                nc.vector.tensor_tensor(
                    out=t1[:], in0=psA, in1=cos_sb[:, sb * 512:(sb + 1) * 512],
                    op=mybir.AluOpType.mult)
                t2 = wk_pool.tile([128, 512], F32, tag="ropeB", name="ropeB", bufs=6)
                nc.vector.tensor_tensor(
                    out=t2[:], in0=psB, in1=sin_sb[:, sb * 512:(sb + 1) * 512],
                    op=mybir.AluOpType.mult)
                nc.vector.tensor_tensor(
                    out=dst[eb][:, sb * 512:(sb + 1) * 512], in0=t1[:], in1=t2[:],
                    op=mybir.AluOpType.add)

            def emit_vproj(st):
                """v for one s-tile, [v_h | 1] layout per head (65 cols)"""
                psV = ps_pool.tile([128, EL], F32, tag="psS", name="pjV")
                for dc in range(DC):
                    nc.tensor.matmul(
                        psV[:], xT128(dc, st), w_sb["wv"][dc][:],
                        start=(dc == 0), stop=(dc == DC - 1))
                nc.vector.memset(v_sb[st][:], 1.0)
                out_ap = v_sb[st][:].rearrange("p (h x) -> p h x", x=65)[:, :, 0:64]
                in_ap = psV[:].rearrange("p (h x) -> p h x", x=64)
                nc.vector.tensor_copy(out_ap, in_ap)

            def emit_attn(sb, hp, weave=None):
                """scores+softmax+attn*v for head pair hp on s-block sb; the
                oT matmuls trail the score matmuls by LAG chunks so the
                in-order PE stream never head-blocks on exp latency"""
                eb = hp
                h0, h1 = 2 * hp, 2 * hp + 1
                ps_o2 = psm_pool.tile([128, 1024], F32, tag="psM", name="ps_o2")
                ps_o0, ps_o1 = ps_o2[0:65, 0:512], ps_o2[0:65, 512:1024]
                pts = {}

                def emit_scores(skc):
                    ps_s = ps_pool.tile([128, 1024], F32, tag="psS", name="ps_s")
                    nc.tensor.matmul(
                        ps_s[:, 0:512],
                        kT[eb][0:64, skc * 128:(skc + 1) * 128],
                        qT[eb][0:64, sb * 512:(sb + 1) * 512],
                        start=True, stop=True)
                    nc.tensor.matmul(
                        ps_s[:, 512:1024],
                        kT[eb][64:128, skc * 128:(skc + 1) * 128],
                        qT[eb][64:128, sb * 512:(sb + 1) * 512],
                        start=True, stop=True)
                    pt = wk_pool.tile([128, 1024], BF16, tag="p_exp", name="p_exp",
                                      bufs=LAG + 3)
                    nc.scalar.activation(pt[:], ps_s[:],
                                         mybir.ActivationFunctionType.Exp,
                                         scale=0.125)
                    pts[skc] = pt

                def emit_ot(skc):
                    pt = pts.pop(skc)
                    nc.tensor.matmul(ps_o0, v_sb[skc][:, h0 * 65:(h0 + 1) * 65],
                                     pt[:, 0:512],
                                     start=(skc == 0), stop=(skc == ST - 1))
                    nc.tensor.matmul(ps_o1, v_sb[skc][:, h1 * 65:(h1 + 1) * 65],
                                     pt[:, 512:1024],
                                     start=(skc == 0), stop=(skc == ST - 1))

                for skc in range(ST):
                    emit_scores(skc)
                    if weave is not None:
                        weave(skc)
                    if skc >= LAG:
                        emit_ot(skc - LAG)
                for skc in range(ST - LAG, ST):
                    emit_ot(skc)

                st_o = wk_pool.tile([128, 512], BF16, tag="oS", name="oS", bufs=4)
                for hi, ps_o in ((0, ps_o0), (1, ps_o1)):
                    l_sb = wk_pool.tile([1, 512], F32, tag="l_sb", name="l_sb")
                    nc.vector.tensor_copy(l_sb[:], ps_o[64:65, :])
                    linv = wk_pool.tile([1, 512], F32, tag="linv", name="linv")
                    nc.vector.reciprocal_approx_fast(out=linv[:], in_=l_sb[:])
                    lrep = wk_pool.tile([64, 512], F32, tag="lrep", name="lrep")
                    nc.gpsimd.partition_broadcast(lrep[:], linv[:])
                    nc.vector.tensor_tensor(
                        out=st_o[hi * 64:(hi + 1) * 64, :], in0=ps_o[0:64, :],
                        in1=lrep[:], op=mybir.AluOpType.mult)
                return st_o

            def emit_y(sb, oS):
                """output projection for s-block sb (normalization is folded
                into oS already)"""
                for stl in range(4):
                    row0 = sb * 512 + stl * 128
                    ps_y = psm_pool.tile([128, D], F32, tag="psM", name="ps_y")
                    for dh in range(2):
                        for hp in range(2):
                            nc.tensor.matmul(
                                ps_y[:, dh * 512:(dh + 1) * 512],
                                oS[hp][:, stl * 128:(stl + 1) * 128],
                                wo_sb[hp][:, dh * 512:(dh + 1) * 512],
                                start=(hp == 0), stop=(hp == 1))
                    y_sb = wk_pool.tile([128, D], F32, tag="y_sb", name="y_sb", bufs=2)
                    nc.vector.tensor_copy(y_sb[:], ps_y[:])
                    nc.sync.dma_start(out_ext.ap()[row0:row0 + 128, :], y_sb[:])

            # ---------- schedule ----------
            # x DMAs first (startup critical path), weights interleaved after
            x_tiles = {st: emit_x_dma(st) for st in range(6)}
            load_w("wk")
            load_w("wk2")
            x_tiles.update({st: emit_x_dma(st) for st in range(6, 8)})
            cos_sb = pp.tile([128, S], F32, tag="cos", name="cos")
            nc.sync.dma_start(cos_sb[:], cos_ext.ap())
            sin_sb = pp.tile([128, S], F32, tag="sin", name="sin")
            nc.sync.dma_start(sin_sb[:], sin_ext.ap())
            load_w("wv")
            for st in range(ST):
                if st + 8 < ST:
                    x_tiles[st + 8] = emit_x_dma(st + 8)
                emit_x_tile(st, x_tiles.pop(st))

            # remaining weights
            load_w("wq")
            load_w("wq2")
            wo_sb = []
            for hp in range(2):
                t = pp.tile([128, D], BF16, tag=f"wo{hp}", name=f"wo{hp}")
                nc.sync.dma_start(t[:], wo_ext.ap()[hp * 128:(hp + 1) * 128, :])
                wo_sb.append(t)

            for sbq in range(SBL):
                emit_proj("wk", "wk2", kT, 0, sbq)
            emit_proj("wq", "wq2", qT, 0, 0)

            # first attention block starts as early as possible; v-proj,
            # k(eb1) and q(eb1,0) are woven between its score chunks
            def weave0(skc):
                emit_vproj(skc)
                if skc % 4 == 2:
                    emit_proj("wk", "wk2", kT, 1, skc // 4)
                if skc == 14:
                    emit_proj("wq", "wq2", qT, 1, 0)

            pending = None
            for sb in range(SBL):
                oS = [emit_attn(sb, 0, weave=weave0 if sb == 0 else None)]
                if sb < SBL - 1:
                    emit_proj("wq", "wq2", qT, 0, sb + 1)
                if pending is not None:
                    emit_y(*pending)
                    pending = None
                oS.append(emit_attn(sb, 1))
                if sb < SBL - 1:
                    emit_proj("wq", "wq2", qT, 1, sb + 1)
                pending = (sb, oS)
            emit_y(*pending)

    nc.compile()
    return nc


def _host_inputs(x, wq, wk, wv, wo):
    """Build per-core input maps (host-side shard + RoPE table prep)."""
    x = np.asarray(x, dtype=np.float32)
    wq = np.asarray(wq, dtype=np.float32)
    wk = np.asarray(wk, dtype=np.float32)
    wv = np.asarray(wv, dtype=np.float32)
    wo = np.asarray(wo, dtype=np.float32)

    def swap_fold(w):  # [D, H, E] -> rope partner weights
        w2 = np.empty_like(w)
        w2[:, :, :E // 2] = -w[:, :, E // 2:]
        w2[:, :, E // 2:] = w[:, :, :E // 2]
        return w2

    wq2, wk2 = swap_fold(wq), swap_fold(wk)

    # rope tables, e-major [128, S]: partition p = (head%2)*64 + j
    pos = np.arange(S, dtype=np.float64)
    j = np.arange(E // 2, dtype=np.float64)
    timescale = MIN_TIMESCALE * (MAX_TIMESCALE / MIN_TIMESCALE) ** (2.0 * j / E)
    sinusoid = pos[None, :] / timescale[:, None]          # [32, S]
    cos32 = np.cos(sinusoid).astype(np.float32)
    sin32 = np.sin(sinusoid).astype(np.float32)
    cos_t = np.tile(cos32, (4, 1))                         # [128, S]
    sin_t = np.tile(sin32, (4, 1))
    ident = np.eye(128, dtype=ml_dtypes.bfloat16)

    bf = ml_dtypes.bfloat16
    in_maps = []
    for c in range(N_CORES):
        b, hg = c // 4, c % 4
        hsl = slice(hg * HL, (hg + 1) * HL)
        in_maps.append({
            "x": np.ascontiguousarray(x[b]),
            "wq": np.ascontiguousarray(wq[:, hsl].reshape(D, EL).astype(bf)),
            "wq2": np.ascontiguousarray(wq2[:, hsl].reshape(D, EL).astype(bf)),
            "wk": np.ascontiguousarray(wk[:, hsl].reshape(D, EL).astype(bf)),
            "wk2": np.ascontiguousarray(wk2[:, hsl].reshape(D, EL).astype(bf)),
            "wv": np.ascontiguousarray(wv[:, hsl].reshape(D, EL).astype(bf)),
            "wo": np.ascontiguousarray(
                wo[hg * EL:(hg + 1) * EL].astype(bf)),
            "cos_t": cos_t,
            "sin_t": sin_t,
            "ident": ident,
        })
    return in_maps


def kernel(x, wq, wk, wv, wo, _trace=False):
    if "nc" not in _CACHE:
        _CACHE["nc"] = _build_graph()
    nc = _CACHE["nc"]
    in_maps = _host_inputs(x, wq, wk, wv, wo)
    kw = {}
    if _trace:
        kw["trace"] = True
    res = run_bass_kernel_spmd(nc, in_maps, list(range(N_CORES)), **kw)
    _CACHE["last_exec_ns"] = res.exec_time_ns
    out = np.zeros((B, S, D), dtype=np.float32)
    for c in range(N_CORES):
        out[c // 4] += res.results[c]["out"]
    return out


# revision 31
# speedup vs baseline: 1.0159x; 1.0159x over previous
"""Distributed Trainium2 kernel for RoPE multi-head attention.

Reference computation (B=2, S=2048, D=1024, H=16, E=64, fp32):
    q = rope(x @ wq); k = rope(x @ wk); v = x @ wv
    o = softmax(q k^T / sqrt(E)) v ; out = o @ wo

Sharding over 8 NeuronCores: core c -> (batch b = c // 4, head group
hg = c % 4 of 4 heads).  Each core computes its heads' attention and a
partial output projection; the host sums the 4 partials per batch
(tensor-parallel unshard).

RoPE is folded into a second projection: rope(q) = (x@wq) * cos_t +
(x@wq2) * sin_t where wq2 has the E-halves swapped/negated (host prep),
so on-device RoPE is 3 elementwise ops against host-built tables.
"""

import sys

for _p in ("/opt/trn_rl_repo", "/opt/pypackages"):
    if _p not in sys.path:
        sys.path.append(_p)

import numpy as np
import ml_dtypes

import concourse.bacc as bacc
import concourse.mybir as mybir
import concourse.tile as tile
from concourse.bass_utils import run_bass_kernel_spmd

F32 = mybir.dt.float32
BF16 = mybir.dt.bfloat16

B, S, D, H, E = 2, 2048, 1024, 16, 64
HL = 4              # heads per core
EL = HL * E         # 256 local e width
N_CORES = 8
MAX_TIMESCALE = 10000.0
MIN_TIMESCALE = 1.0

_CACHE = {}


def _build_graph():
    nc = bacc.Bacc("TRN2", target_bir_lowering=False, debug=False,
                   num_devices=N_CORES)

    x_ext = nc.dram_tensor("x", [S, D], F32, kind="ExternalInput")
    w_ext = {
        name: nc.dram_tensor(name, [D, EL], BF16, kind="ExternalInput")
        for name in ("wq", "wq2", "wk", "wk2", "wv")
    }
    wo_ext = nc.dram_tensor("wo", [EL, D], BF16, kind="ExternalInput")
    cos_ext = nc.dram_tensor("cos_t", [128, S], F32, kind="ExternalInput")
    sin_ext = nc.dram_tensor("sin_t", [128, S], F32, kind="ExternalInput")
    id_ext = nc.dram_tensor("ident", [128, 128], BF16, kind="ExternalInput")
    out_ext = nc.dram_tensor("out", [S, D], F32, kind="ExternalOutput")

    ST = S // 128       # 16 s-tiles
    SBL = S // 512      # 4 s-blocks
    DC = D // 128       # 8 d-chunks

    with tile.TileContext(nc) as tc:
        with (
            tc.tile_pool(name="persist", bufs=1) as pp,
            tc.tile_pool(name="xin", bufs=3) as xin_pool,
            tc.tile_pool(name="work", bufs=4) as wk_pool,
            tc.tile_pool(name="ps", bufs=2, space="PSUM") as ps_pool,
            tc.tile_pool(name="psm", bufs=2, space="PSUM") as psm_pool,
        ):
            LAG = 3

            # ---------- constants / weights ----------
            ident = pp.tile([128, 128], BF16, tag="ident", name="ident")
            nc.sync.dma_start(ident[:], id_ext.ap())
            w_sb = {}

            def load_w(name):
                tiles = []
                for dc in range(DC):
                    t = pp.tile([128, EL], BF16, tag=f"{name}{dc}", name=f"{name}{dc}")
                    nc.sync.dma_start(t[:], w_ext[name].ap()[dc * 128:(dc + 1) * 128, :])
                    tiles.append(t)
                w_sb[name] = tiles


            # ---------- persistent tiles ----------
            # xT split into 4 s-quarters; xT_q[q][:, dc*512+s] = x[q*512+s, dc*128+p]
            xT_q = [pp.tile([128, DC * 512], BF16, tag=f"xTq{q}", name=f"xTq{q}")
                    for q in range(4)]

            def xT(dc, sb):
                return xT_q[sb][:, dc * 512:(dc + 1) * 512]

            def xT128(dc, st):
                q, r = st // 4, st % 4
                return xT_q[q][:, dc * 512 + r * 128:dc * 512 + (r + 1) * 128]

            qT = [pp.tile([128, S], BF16, tag=f"qT{eb}", name=f"qT{eb}") for eb in range(2)]
            kT = [pp.tile([128, S], BF16, tag=f"kT{eb}", name=f"kT{eb}") for eb in range(2)]
            v_sb = [pp.tile([128, HL * 65], BF16, tag=f"v{st}", name=f"v{st}")
                    for st in range(ST)]

            # ---------- emitters ----------
            def emit_x_dma(st):
                x_t = xin_pool.tile([128, D], F32, tag="x_in", name="x_in",
                                    bufs=8)
                nc.sync.dma_start(x_t[:], x_ext.ap()[st * 128:(st + 1) * 128, :])
                return x_t

            def emit_x_tile(st, x_t):
                """cast bf16 (alternating ACT/DVE), PE-transpose into xT_q"""
                xb = xin_pool.tile([128, D], BF16, tag="x_bf", name="x_bf")
                if st % 2 == 0:
                    nc.scalar.copy(xb[:], x_t[:])
                else:
                    nc.vector.tensor_copy(xb[:], x_t[:])
                ps_t = ps_pool.tile([128, D], BF16, tag="psS", name="tp")
                for dc in range(DC):
                    nc.tensor.matmul(ps_t[:, dc * 128:(dc + 1) * 128],
                                     xb[:, dc * 128:(dc + 1) * 128], ident[:],
                                     is_transpose=True, start=(dc == 0),
                                     stop=(dc == DC - 1))
                q, r = st // 4, st % 4
                dst = xT_q[q][:].rearrange("p (d s) -> p d s", s=512)[:, :, r * 128:(r + 1) * 128]
                src_ap = ps_t[:].rearrange("p (d c) -> p d c", c=128)
                nc.vector.tensor_copy(dst, src_ap)

            def emit_proj(w1n, w2n, dst, eb, sb):
                """rope-folded projection of one [128, 512] block of qT/kT"""
                pj = psm_pool.tile([128, 1024], F32, tag="psM", name="pj")
                psA, psB = pj[:, 0:512], pj[:, 512:1024]
                for dc in range(DC):
                    nc.tensor.matmul(
                        psA, w_sb[w1n][dc][:, eb * 128:(eb + 1) * 128],
                        xT(dc, sb), start=(dc == 0), stop=(dc == DC - 1))
                for dc in range(DC):
                    nc.tensor.matmul(
                        psB, w_sb[w2n][dc][:, eb * 128:(eb + 1) * 128],
                        xT(dc, sb), start=(dc == 0), stop=(dc == DC - 1))
                t1 = wk_pool.tile([128, 512], F32, tag="ropeA", name="ropeA", bufs=6)
                nc.vector.tensor_tensor(
                    out=t1[:], in0=psA, in1=cos_sb[:, sb * 512:(sb + 1) * 512],
                    op=mybir.AluOpType.mult)
                t2 = wk_pool.tile([128, 512], F32, tag="ropeB", name="ropeB", bufs=6)
                nc.vector.tensor_tensor(
                    out=t2[:], in0=psB, in1=sin_sb[:, sb * 512:(sb + 1) * 512],
                    op=mybir.AluOpType.mult)
                nc.vector.tensor_tensor(
                    out=dst[eb][:, sb * 512:(sb + 1) * 512], in0=t1[:], in1=t2[:],
                    op=mybir.AluOpType.add)

            def emit_vproj(st):
                """v for one s-tile, [v_h | 1] layout per head (65 cols)"""
                psV = ps_pool.tile([128, EL], F32, tag="psS", name="pjV")
                for dc in range(DC):
                    nc.tensor.matmul(
                        psV[:], xT128(dc, st), w_sb["wv"][dc][:],
                        start=(dc == 0), stop=(dc == DC - 1))
                nc.vector.memset(v_sb[st][:], 1.0)
                out_ap = v_sb[st][:].rearrange("p (h x) -> p h x", x=65)[:, :, 0:64]
                in_ap = psV[:].rearrange("p (h x) -> p h x", x=64)
                nc.vector.tensor_copy(out_ap, in_ap)

            def emit_attn(sb, hp, weave=None):
                """scores+softmax+attn*v for head pair hp on s-block sb; the
                oT matmuls trail the score matmuls by LAG chunks so the
                in-order PE stream never head-blocks on exp latency"""
                eb = hp
                h0, h1 = 2 * hp, 2 * hp + 1
                ps_o2 = psm_pool.tile([128, 1024], F32, tag="psM", name="ps_o2")
                ps_o0, ps_o1 = ps_o2[0:65, 0:512], ps_o2[0:65, 512:1024]
                pts = {}

                def emit_scores(skc):
                    ps_s = ps_pool.tile([128, 1024], F32, tag="psS", name="ps_s")
                    nc.tensor.matmul(
                        ps_s[:, 0:512],
                        kT[eb][0:64, skc * 128:(skc + 1) * 128],
                        qT[eb][0:64, sb * 512:(sb + 1) * 512],
                        start=True, stop=True)
                    nc.tensor.matmul(
                        ps_s[:, 512:1024],
                        kT[eb][64:128, skc * 128:(skc + 1) * 128],
                        qT[eb][64:128, sb * 512:(sb + 1) * 512],
                        start=True, stop=True)
                    pt = wk_pool.tile([128, 1024], BF16, tag="p_exp", name="p_exp",
                                      bufs=LAG + 3)
                    nc.scalar.activation(pt[:], ps_s[:],
                                         mybir.ActivationFunctionType.Exp,
                                         scale=0.125)
                    pts[skc] = pt

                def emit_ot(skc):
                    pt = pts.pop(skc)
                    nc.tensor.matmul(ps_o0, v_sb[skc][:, h0 * 65:(h0 + 1) * 65],
                                     pt[:, 0:512],
                                     start=(skc == 0), stop=(skc == ST - 1))
                    nc.tensor.matmul(ps_o1, v_sb[skc][:, h1 * 65:(h1 + 1) * 65],
                                     pt[:, 512:1024],
                                     start=(skc == 0), stop=(skc == ST - 1))

                for skc in range(ST):
                    emit_scores(skc)
                    if weave is not None:
                        weave(skc)
                    if skc >= LAG:
                        emit_ot(skc - LAG)
                for skc in range(ST - LAG, ST):
                    emit_ot(skc)

                st_o = wk_pool.tile([128, 512], BF16, tag="oS", name="oS", bufs=4)
                for hi, ps_o in ((0, ps_o0), (1, ps_o1)):
                    l_sb = wk_pool.tile([1, 512], F32, tag="l_sb", name="l_sb")
                    nc.vector.tensor_copy(l_sb[:], ps_o[64:65, :])
                    linv = wk_pool.tile([1, 512], F32, tag="linv", name="linv")
                    nc.vector.reciprocal_approx_fast(out=linv[:], in_=l_sb[:])
                    lrep = wk_pool.tile([64, 512], F32, tag="lrep", name="lrep")
                    nc.gpsimd.partition_broadcast(lrep[:], linv[:])
                    nc.vector.tensor_tensor(
                        out=st_o[hi * 64:(hi + 1) * 64, :], in0=ps_o[0:64, :],
                        in1=lrep[:], op=mybir.AluOpType.mult)
                return st_o

            def emit_y(sb, oS):
                """output projection for s-block sb (normalization is folded
                into oS already)"""
                for stl in range(4):
                    row0 = sb * 512 + stl * 128
                    ps_y = psm_pool.tile([128, D], F32, tag="psM", name="ps_y")
                    for dh in range(2):
                        for hp in range(2):
                            nc.tensor.matmul(
                                ps_y[:, dh * 512:(dh + 1) * 512],
                                oS[hp][:, stl * 128:(stl + 1) * 128],
                                wo_sb[hp][:, dh * 512:(dh + 1) * 512],
                                start=(hp == 0), stop=(hp == 1))
                    y_sb = wk_pool.tile([128, D], F32, tag="y_sb", name="y_sb", bufs=2)
                    nc.vector.tensor_copy(y_sb[:], ps_y[:])
                    nc.sync.dma_start(out_ext.ap()[row0:row0 + 128, :], y_sb[:])

            # ---------- schedule ----------
            # x DMAs first (startup critical path), weights interleaved after
            x_tiles = {st: emit_x_dma(st) for st in range(6)}
            load_w("wk")
            load_w("wk2")
            x_tiles.update({st: emit_x_dma(st) for st in range(6, 8)})
            cos_sb = pp.tile([128, S], F32, tag="cos", name="cos")
            nc.sync.dma_start(cos_sb[:], cos_ext.ap())
            sin_sb = pp.tile([128, S], F32, tag="sin", name="sin")
            nc.sync.dma_start(sin_sb[:], sin_ext.ap())
            load_w("wv")
            for st in range(ST):
                if st + 8 < ST:
                    x_tiles[st + 8] = emit_x_dma(st + 8)
                emit_x_tile(st, x_tiles.pop(st))

            # remaining weights
            load_w("wq")
            load_w("wq2")
            wo_sb = []
            for hp in range(2):
                t = pp.tile([128, D], BF16, tag=f"wo{hp}", name=f"wo{hp}")
                nc.sync.dma_start(t[:], wo_ext.ap()[hp * 128:(hp + 1) * 128, :])
                wo_sb.append(t)

            for sbq in range(SBL):
                emit_proj("wk", "wk2", kT, 0, sbq)
            emit_proj("wq", "wq2", qT, 0, 0)

            # first attention block starts as early as possible; v-proj,
            # k(eb1) and q(eb1,0) are woven between its score chunks
            def weave0(skc):
                emit_vproj(skc)
                if skc % 4 == 2:
                    emit_proj("wk", "wk2", kT, 1, skc // 4)
                if skc == 14:
                    emit_proj("wq", "wq2", qT, 1, 0)

            pending = None
            for sb in range(SBL):
                oS = [emit_attn(sb, 0, weave=weave0 if sb == 0 else None)]
                if sb < SBL - 1:
                    emit_proj("wq", "wq2", qT, 0, sb + 1)
                if pending is not None:
                    emit_y(*pending)
                    pending = None
                oS.append(emit_attn(sb, 1))
                if sb < SBL - 1:
                    emit_proj("wq", "wq2", qT, 1, sb + 1)
                pending = (sb, oS)
            emit_y(*pending)

    nc.compile()
    return nc


def _host_inputs(x, wq, wk, wv, wo):
    """Build per-core input maps (host-side shard + RoPE table prep)."""
    x = np.asarray(x, dtype=np.float32)
    wq = np.asarray(wq, dtype=np.float32)
    wk = np.asarray(wk, dtype=np.float32)
    wv = np.asarray(wv, dtype=np.float32)
    wo = np.asarray(wo, dtype=np.float32)

    def swap_fold(w):  # [D, H, E] -> rope partner weights
        w2 = np.empty_like(w)
        w2[:, :, :E // 2] = -w[:, :, E // 2:]
        w2[:, :, E // 2:] = w[:, :, :E // 2]
        return w2

    wq2, wk2 = swap_fold(wq), swap_fold(wk)

    # rope tables, e-major [128, S]: partition p = (head%2)*64 + j
    pos = np.arange(S, dtype=np.float64)
    j = np.arange(E // 2, dtype=np.float64)
    timescale = MIN_TIMESCALE * (MAX_TIMESCALE / MIN_TIMESCALE) ** (2.0 * j / E)
    sinusoid = pos[None, :] / timescale[:, None]          # [32, S]
    cos32 = np.cos(sinusoid).astype(np.float32)
    sin32 = np.sin(sinusoid).astype(np.float32)
    cos_t = np.tile(cos32, (4, 1))                         # [128, S]
    sin_t = np.tile(sin32, (4, 1))
    ident = np.eye(128, dtype=ml_dtypes.bfloat16)

    bf = ml_dtypes.bfloat16
    in_maps = []
    for c in range(N_CORES):
        b, hg = c // 4, c % 4
        hsl = slice(hg * HL, (hg + 1) * HL)
        in_maps.append({
            "x": np.ascontiguousarray(x[b]),
            "wq": np.ascontiguousarray(wq[:, hsl].reshape(D, EL).astype(bf)),
            "wq2": np.ascontiguousarray(wq2[:, hsl].reshape(D, EL).astype(bf)),
            "wk": np.ascontiguousarray(wk[:, hsl].reshape(D, EL).astype(bf)),
            "wk2": np.ascontiguousarray(wk2[:, hsl].reshape(D, EL).astype(bf)),
            "wv": np.ascontiguousarray(wv[:, hsl].reshape(D, EL).astype(bf)),
            "wo": np.ascontiguousarray(
                wo[hg * EL:(hg + 1) * EL].astype(bf)),
            "cos_t": cos_t,
            "sin_t": sin_t,
            "ident": ident,
        })
    return in_maps


def kernel(x, wq, wk, wv, wo, _trace=False):
    if "nc" not in _CACHE:
        _CACHE["nc"] = _build_graph()
    nc = _CACHE["nc"]
    in_maps = _host_inputs(x, wq, wk, wv, wo)
    kw = {}
    if _trace:
        kw["trace"] = True
    res = run_bass_kernel_spmd(nc, in_maps, list(range(N_CORES)), **kw)
    _CACHE["last_exec_ns"] = res.exec_time_ns
    out = np.zeros((B, S, D), dtype=np.float32)
    for c in range(N_CORES):
        out[c // 4] += res.results[c]["out"]
    return out


# revision 32
# speedup vs baseline: 1.0312x; 1.0151x over previous
"""Distributed Trainium2 kernel for RoPE multi-head attention.

Reference computation (B=2, S=2048, D=1024, H=16, E=64, fp32):
    q = rope(x @ wq); k = rope(x @ wk); v = x @ wv
    o = softmax(q k^T / sqrt(E)) v ; out = o @ wo

Sharding over 8 NeuronCores: core c -> (batch b = c // 4, head group
hg = c % 4 of 4 heads).  Each core computes its heads' attention and a
partial output projection; the host sums the 4 partials per batch
(tensor-parallel unshard).

RoPE is folded into a second projection: rope(q) = (x@wq) * cos_t +
(x@wq2) * sin_t where wq2 has the E-halves swapped/negated (host prep),
so on-device RoPE is 3 elementwise ops against host-built tables.
"""

import sys

for _p in ("/opt/trn_rl_repo", "/opt/pypackages"):
    if _p not in sys.path:
        sys.path.append(_p)

import numpy as np
import ml_dtypes

import concourse.bacc as bacc
import concourse.mybir as mybir
import concourse.tile as tile
from concourse.bass_utils import run_bass_kernel_spmd

F32 = mybir.dt.float32
BF16 = mybir.dt.bfloat16

B, S, D, H, E = 2, 2048, 1024, 16, 64
HL = 4              # heads per core
EL = HL * E         # 256 local e width
N_CORES = 8
MAX_TIMESCALE = 10000.0
MIN_TIMESCALE = 1.0

_CACHE = {}


def _build_graph():
    nc = bacc.Bacc("TRN2", target_bir_lowering=False, debug=False,
                   num_devices=N_CORES)

    x_ext = nc.dram_tensor("x", [S, D], F32, kind="ExternalInput")
    w_ext = {
        name: nc.dram_tensor(name, [D, EL], BF16, kind="ExternalInput")
        for name in ("wq", "wq2", "wk", "wk2", "wv")
    }
    wo_ext = nc.dram_tensor("wo", [EL, D], BF16, kind="ExternalInput")
    cos_ext = nc.dram_tensor("cos_t", [128, S], F32, kind="ExternalInput")
    sin_ext = nc.dram_tensor("sin_t", [128, S], F32, kind="ExternalInput")
    id_ext = nc.dram_tensor("ident", [128, 128], BF16, kind="ExternalInput")
    out_ext = nc.dram_tensor("out", [S, D], F32, kind="ExternalOutput")

    ST = S // 128       # 16 s-tiles
    SBL = S // 512      # 4 s-blocks
    DC = D // 128       # 8 d-chunks

    with tile.TileContext(nc) as tc:
        with (
            tc.tile_pool(name="persist", bufs=1) as pp,
            tc.tile_pool(name="xin", bufs=3) as xin_pool,
            tc.tile_pool(name="work", bufs=4) as wk_pool,
            tc.tile_pool(name="ps", bufs=2, space="PSUM") as ps_pool,
            tc.tile_pool(name="psm", bufs=2, space="PSUM") as psm_pool,
        ):
            LAG = 3

            # ---------- constants / weights ----------
            ident = pp.tile([128, 128], BF16, tag="ident", name="ident")
            nc.sync.dma_start(ident[:], id_ext.ap())
            w_sb = {}

            def load_w(name):
                tiles = []
                for dc in range(DC):
                    t = pp.tile([128, EL], BF16, tag=f"{name}{dc}", name=f"{name}{dc}")
                    nc.sync.dma_start(t[:], w_ext[name].ap()[dc * 128:(dc + 1) * 128, :])
                    tiles.append(t)
                w_sb[name] = tiles


            # ---------- persistent tiles ----------
            # xT split into 4 s-quarters; xT_q[q][:, dc*512+s] = x[q*512+s, dc*128+p]
            xT_q = [pp.tile([128, DC * 512], BF16, tag=f"xTq{q}", name=f"xTq{q}")
                    for q in range(4)]

            def xT(dc, sb):
                return xT_q[sb][:, dc * 512:(dc + 1) * 512]

            def xT128(dc, st):
                q, r = st // 4, st % 4
                return xT_q[q][:, dc * 512 + r * 128:dc * 512 + (r + 1) * 128]

            qT = [pp.tile([128, S], BF16, tag=f"qT{eb}", name=f"qT{eb}") for eb in range(2)]
            kT = [pp.tile([128, S], BF16, tag=f"kT{eb}", name=f"kT{eb}") for eb in range(2)]
            v_sb = [pp.tile([128, HL * 65], BF16, tag=f"v{st}", name=f"v{st}")
                    for st in range(ST)]

            # ---------- emitters ----------
            def emit_x_dma(st):
                x_t = xin_pool.tile([128, D], F32, tag="x_in", name="x_in",
                                    bufs=8)
                nc.sync.dma_start(x_t[:], x_ext.ap()[st * 128:(st + 1) * 128, :])
                return x_t

            def emit_x_tile(st, x_t):
                """cast bf16 (alternating ACT/DVE), PE-transpose into xT_q"""
                xb = xin_pool.tile([128, D], BF16, tag="x_bf", name="x_bf")
                if st % 2 == 0:
                    nc.scalar.copy(xb[:], x_t[:])
                else:
                    nc.vector.tensor_copy(xb[:], x_t[:])
                ps_t = ps_pool.tile([128, D], BF16, tag="psS", name="tp")
                for dc in range(DC):
                    nc.tensor.matmul(ps_t[:, dc * 128:(dc + 1) * 128],
                                     xb[:, dc * 128:(dc + 1) * 128], ident[:],
                                     is_transpose=True, start=(dc == 0),
                                     stop=(dc == DC - 1))
                q, r = st // 4, st % 4
                dst = xT_q[q][:].rearrange("p (d s) -> p d s", s=512)[:, :, r * 128:(r + 1) * 128]
                src_ap = ps_t[:].rearrange("p (d c) -> p d c", c=128)
                nc.vector.tensor_copy(dst, src_ap)

            def emit_proj(w1n, w2n, dst, eb, sb):
                """rope-folded projection of one [128, 512] block of qT/kT"""
                pj = psm_pool.tile([128, 1024], F32, tag="psM", name="pj")
                psA, psB = pj[:, 0:512], pj[:, 512:1024]
                for dc in range(DC):
                    nc.tensor.matmul(
                        psA, w_sb[w1n][dc][:, eb * 128:(eb + 1) * 128],
                        xT(dc, sb), start=(dc == 0), stop=(dc == DC - 1))
                for dc in range(DC):
                    nc.tensor.matmul(
                        psB, w_sb[w2n][dc][:, eb * 128:(eb + 1) * 128],
                        xT(dc, sb), start=(dc == 0), stop=(dc == DC - 1))
                t1 = wk_pool.tile([128, 512], F32, tag="ropeA", name="ropeA", bufs=6)
                nc.vector.tensor_tensor(
                    out=t1[:], in0=psA, in1=cos_sb[:, sb * 512:(sb + 1) * 512],
                    op=mybir.AluOpType.mult)
                t2 = wk_pool.tile([128, 512], F32, tag="ropeB", name="ropeB", bufs=6)
                nc.vector.tensor_tensor(
                    out=t2[:], in0=psB, in1=sin_sb[:, sb * 512:(sb + 1) * 512],
                    op=mybir.AluOpType.mult)
                nc.vector.tensor_tensor(
                    out=dst[eb][:, sb * 512:(sb + 1) * 512], in0=t1[:], in1=t2[:],
                    op=mybir.AluOpType.add)

            def emit_vproj(st):
                """v for one s-tile, [v_h | 1] layout per head (65 cols)"""
                psV = ps_pool.tile([128, EL], F32, tag="psS", name="pjV")
                for dc in range(DC):
                    nc.tensor.matmul(
                        psV[:], xT128(dc, st), w_sb["wv"][dc][:],
                        start=(dc == 0), stop=(dc == DC - 1))
                nc.vector.memset(v_sb[st][:], 1.0)
                out_ap = v_sb[st][:].rearrange("p (h x) -> p h x", x=65)[:, :, 0:64]
                in_ap = psV[:].rearrange("p (h x) -> p h x", x=64)
                nc.vector.tensor_copy(out_ap, in_ap)

            def emit_attn(sb, hp, weave=None):
                """scores+softmax+attn*v for head pair hp on s-block sb; the
                oT matmuls trail the score matmuls by LAG chunks so the
                in-order PE stream never head-blocks on exp latency"""
                eb = hp
                h0, h1 = 2 * hp, 2 * hp + 1
                ps_o2 = psm_pool.tile([128, 1024], F32, tag="psM", name="ps_o2")
                ps_o0, ps_o1 = ps_o2[0:65, 0:512], ps_o2[0:65, 512:1024]
                pts = {}

                def emit_scores(skc):
                    ps_s = ps_pool.tile([128, 1024], F32, tag="psS", name="ps_s")
                    nc.tensor.matmul(
                        ps_s[:, 0:512],
                        kT[eb][0:64, skc * 128:(skc + 1) * 128],
                        qT[eb][0:64, sb * 512:(sb + 1) * 512],
                        start=True, stop=True)
                    nc.tensor.matmul(
                        ps_s[:, 512:1024],
                        kT[eb][64:128, skc * 128:(skc + 1) * 128],
                        qT[eb][64:128, sb * 512:(sb + 1) * 512],
                        start=True, stop=True)
                    pt = wk_pool.tile([128, 1024], BF16, tag="p_exp", name="p_exp",
                                      bufs=LAG + 3)
                    nc.scalar.activation(pt[:], ps_s[:],
                                         mybir.ActivationFunctionType.Exp,
                                         scale=0.125)
                    pts[skc] = pt

                def emit_ot(skc):
                    pt = pts.pop(skc)
                    nc.tensor.matmul(ps_o0, v_sb[skc][:, h0 * 65:(h0 + 1) * 65],
                                     pt[:, 0:512],
                                     start=(skc == 0), stop=(skc == ST - 1))
                    nc.tensor.matmul(ps_o1, v_sb[skc][:, h1 * 65:(h1 + 1) * 65],
                                     pt[:, 512:1024],
                                     start=(skc == 0), stop=(skc == ST - 1))

                for skc in range(ST):
                    emit_scores(skc)
                    if weave is not None:
                        weave(skc)
                    if skc >= LAG:
                        emit_ot(skc - LAG)
                for skc in range(ST - LAG, ST):
                    emit_ot(skc)

                # normalize both heads with the two chains interleaved so the
                # DVE works during the gpsimd broadcast of the other head
                st_o = wk_pool.tile([128, 512], BF16, tag="oS", name="oS", bufs=4)
                lreps = []
                for hi, ps_o in ((0, ps_o0), (1, ps_o1)):
                    l_sb = wk_pool.tile([1, 512], F32, tag="l_sb", name="l_sb")
                    nc.vector.tensor_copy(l_sb[:], ps_o[64:65, :])
                    linv = wk_pool.tile([1, 512], F32, tag="linv", name="linv")
                    nc.vector.reciprocal_approx_fast(out=linv[:], in_=l_sb[:])
                    lrep = wk_pool.tile([64, 512], F32, tag="lrep", name="lrep")
                    nc.gpsimd.partition_broadcast(lrep[:], linv[:])
                    lreps.append(lrep)
                for hi, ps_o in ((0, ps_o0), (1, ps_o1)):
                    nc.vector.tensor_tensor(
                        out=st_o[hi * 64:(hi + 1) * 64, :], in0=ps_o[0:64, :],
                        in1=lreps[hi][:], op=mybir.AluOpType.mult)
                return st_o

            def emit_y(sb, oS):
                """output projection for s-block sb (normalization is folded
                into oS already)"""
                for stl in range(4):
                    row0 = sb * 512 + stl * 128
                    ps_y = psm_pool.tile([128, D], F32, tag="psM", name="ps_y")
                    for dh in range(2):
                        for hp in range(2):
                            nc.tensor.matmul(
                                ps_y[:, dh * 512:(dh + 1) * 512],
                                oS[hp][:, stl * 128:(stl + 1) * 128],
                                wo_sb[hp][:, dh * 512:(dh + 1) * 512],
                                start=(hp == 0), stop=(hp == 1))
                    y_sb = wk_pool.tile([128, D], F32, tag="y_sb", name="y_sb", bufs=2)
                    nc.vector.tensor_copy(y_sb[:], ps_y[:])
                    nc.sync.dma_start(out_ext.ap()[row0:row0 + 128, :], y_sb[:])

            # ---------- schedule ----------
            # x DMAs first (startup critical path), weights interleaved after
            x_tiles = {st: emit_x_dma(st) for st in range(6)}
            load_w("wk")
            load_w("wk2")
            x_tiles.update({st: emit_x_dma(st) for st in range(6, 8)})
            cos_sb = pp.tile([128, S], F32, tag="cos", name="cos")
            nc.sync.dma_start(cos_sb[:], cos_ext.ap())
            sin_sb = pp.tile([128, S], F32, tag="sin", name="sin")
            nc.sync.dma_start(sin_sb[:], sin_ext.ap())
            load_w("wv")
            for st in range(ST):
                if st + 8 < ST:
                    x_tiles[st + 8] = emit_x_dma(st + 8)
                emit_x_tile(st, x_tiles.pop(st))

            # remaining weights
            load_w("wq")
            load_w("wq2")
            wo_sb = []
            for hp in range(2):
                t = pp.tile([128, D], BF16, tag=f"wo{hp}", name=f"wo{hp}")
                nc.sync.dma_start(t[:], wo_ext.ap()[hp * 128:(hp + 1) * 128, :])
                wo_sb.append(t)

            for sbq in range(SBL):
                emit_proj("wk", "wk2", kT, 0, sbq)
            emit_proj("wq", "wq2", qT, 0, 0)

            # first attention block starts as early as possible; v-proj,
            # k(eb1) and q(eb1,0) are woven between its score chunks
            def weave0(skc):
                emit_vproj(skc)
                if skc % 4 == 2:
                    emit_proj("wk", "wk2", kT, 1, skc // 4)
                if skc == 14:
                    emit_proj("wq", "wq2", qT, 1, 0)

            pending = None
            for sb in range(SBL):
                oS = [emit_attn(sb, 0, weave=weave0 if sb == 0 else None)]
                if sb < SBL - 1:
                    emit_proj("wq", "wq2", qT, 0, sb + 1)
                if pending is not None:
                    emit_y(*pending)
                    pending = None
                oS.append(emit_attn(sb, 1))
                if sb < SBL - 1:
                    emit_proj("wq", "wq2", qT, 1, sb + 1)
                pending = (sb, oS)
            emit_y(*pending)

    nc.compile()
    return nc


def _host_inputs(x, wq, wk, wv, wo):
    """Build per-core input maps (host-side shard + RoPE table prep)."""
    x = np.asarray(x, dtype=np.float32)
    wq = np.asarray(wq, dtype=np.float32)
    wk = np.asarray(wk, dtype=np.float32)
    wv = np.asarray(wv, dtype=np.float32)
    wo = np.asarray(wo, dtype=np.float32)

    def swap_fold(w):  # [D, H, E] -> rope partner weights
        w2 = np.empty_like(w)
        w2[:, :, :E // 2] = -w[:, :, E // 2:]
        w2[:, :, E // 2:] = w[:, :, :E // 2]
        return w2

    wq2, wk2 = swap_fold(wq), swap_fold(wk)

    # rope tables, e-major [128, S]: partition p = (head%2)*64 + j
    pos = np.arange(S, dtype=np.float64)
    j = np.arange(E // 2, dtype=np.float64)
    timescale = MIN_TIMESCALE * (MAX_TIMESCALE / MIN_TIMESCALE) ** (2.0 * j / E)
    sinusoid = pos[None, :] / timescale[:, None]          # [32, S]
    cos32 = np.cos(sinusoid).astype(np.float32)
    sin32 = np.sin(sinusoid).astype(np.float32)
    cos_t = np.tile(cos32, (4, 1))                         # [128, S]
    sin_t = np.tile(sin32, (4, 1))
    ident = np.eye(128, dtype=ml_dtypes.bfloat16)

    bf = ml_dtypes.bfloat16
    in_maps = []
    for c in range(N_CORES):
        b, hg = c // 4, c % 4
        hsl = slice(hg * HL, (hg + 1) * HL)
        in_maps.append({
            "x": np.ascontiguousarray(x[b]),
            "wq": np.ascontiguousarray(wq[:, hsl].reshape(D, EL).astype(bf)),
            "wq2": np.ascontiguousarray(wq2[:, hsl].reshape(D, EL).astype(bf)),
            "wk": np.ascontiguousarray(wk[:, hsl].reshape(D, EL).astype(bf)),
            "wk2": np.ascontiguousarray(wk2[:, hsl].reshape(D, EL).astype(bf)),
            "wv": np.ascontiguousarray(wv[:, hsl].reshape(D, EL).astype(bf)),
            "wo": np.ascontiguousarray(
                wo[hg * EL:(hg + 1) * EL].astype(bf)),
            "cos_t": cos_t,
            "sin_t": sin_t,
            "ident": ident,
        })
    return in_maps


def kernel(x, wq, wk, wv, wo, _trace=False):
    if "nc" not in _CACHE:
        _CACHE["nc"] = _build_graph()
    nc = _CACHE["nc"]
    in_maps = _host_inputs(x, wq, wk, wv, wo)
    kw = {}
    if _trace:
        kw["trace"] = True
    res = run_bass_kernel_spmd(nc, in_maps, list(range(N_CORES)), **kw)
    _CACHE["last_exec_ns"] = res.exec_time_ns
    out = np.zeros((B, S, D), dtype=np.float32)
    for c in range(N_CORES):
        out[c // 4] += res.results[c]["out"]
    return out


# revision 34
# speedup vs baseline: 1.0336x; 1.0024x over previous
"""Distributed Trainium2 kernel for RoPE multi-head attention.

Reference computation (B=2, S=2048, D=1024, H=16, E=64, fp32):
    q = rope(x @ wq); k = rope(x @ wk); v = x @ wv
    o = softmax(q k^T / sqrt(E)) v ; out = o @ wo

Sharding over 8 NeuronCores: core c -> (batch b = c // 4, head group
hg = c % 4 of 4 heads).  Each core computes its heads' attention and a
partial output projection; the host sums the 4 partials per batch
(tensor-parallel unshard).

RoPE is folded into a second projection: rope(q) = (x@wq) * cos_t +
(x@wq2) * sin_t where wq2 has the E-halves swapped/negated (host prep),
so on-device RoPE is 3 elementwise ops against host-built tables.
"""

import sys

for _p in ("/opt/trn_rl_repo", "/opt/pypackages"):
    if _p not in sys.path:
        sys.path.append(_p)

import numpy as np
import ml_dtypes

import concourse.bacc as bacc
import concourse.mybir as mybir
import concourse.tile as tile
from concourse.bass_utils import run_bass_kernel_spmd

F32 = mybir.dt.float32
BF16 = mybir.dt.bfloat16

B, S, D, H, E = 2, 2048, 1024, 16, 64
HL = 4              # heads per core
EL = HL * E         # 256 local e width
N_CORES = 8
MAX_TIMESCALE = 10000.0
MIN_TIMESCALE = 1.0

_CACHE = {}


def _build_graph():
    nc = bacc.Bacc("TRN2", target_bir_lowering=False, debug=False,
                   num_devices=N_CORES)

    x_ext = nc.dram_tensor("x", [S, D], F32, kind="ExternalInput")
    w_ext = {
        name: nc.dram_tensor(name, [D, EL], BF16, kind="ExternalInput")
        for name in ("wq", "wq2", "wk", "wk2", "wv")
    }
    wo_ext = nc.dram_tensor("wo", [EL, D], BF16, kind="ExternalInput")
    cos_ext = nc.dram_tensor("cos_t", [128, S], F32, kind="ExternalInput")
    sin_ext = nc.dram_tensor("sin_t", [128, S], F32, kind="ExternalInput")
    id_ext = nc.dram_tensor("ident", [128, 128], BF16, kind="ExternalInput")
    out_ext = nc.dram_tensor("out", [S, D], F32, kind="ExternalOutput")

    ST = S // 128       # 16 s-tiles
    SBL = S // 512      # 4 s-blocks
    DC = D // 128       # 8 d-chunks

    with tile.TileContext(nc) as tc:
        with (
            tc.tile_pool(name="persist", bufs=1) as pp,
            tc.tile_pool(name="xin", bufs=3) as xin_pool,
            tc.tile_pool(name="work", bufs=4) as wk_pool,
            tc.tile_pool(name="ps", bufs=2, space="PSUM") as ps_pool,
            tc.tile_pool(name="psm", bufs=2, space="PSUM") as psm_pool,
        ):
            LAG = 3

            # ---------- constants / weights ----------
            ident = pp.tile([128, 128], BF16, tag="ident", name="ident")
            nc.sync.dma_start(ident[:], id_ext.ap())
            w_sb = {}

            def load_w(name):
                tiles = []
                for dc in range(DC):
                    t = pp.tile([128, EL], BF16, tag=f"{name}{dc}", name=f"{name}{dc}")
                    nc.sync.dma_start(t[:], w_ext[name].ap()[dc * 128:(dc + 1) * 128, :])
                    tiles.append(t)
                w_sb[name] = tiles


            # ---------- persistent tiles ----------
            # xT split into 4 s-quarters; xT_q[q][:, dc*512+s] = x[q*512+s, dc*128+p]
            xT_q = [pp.tile([128, DC * 512], BF16, tag=f"xTq{q}", name=f"xTq{q}")
                    for q in range(4)]

            def xT(dc, sb):
                return xT_q[sb][:, dc * 512:(dc + 1) * 512]

            def xT128(dc, st):
                q, r = st // 4, st % 4
                return xT_q[q][:, dc * 512 + r * 128:dc * 512 + (r + 1) * 128]

            qT = [pp.tile([128, S], BF16, tag=f"qT{eb}", name=f"qT{eb}") for eb in range(2)]
            kT = [pp.tile([128, S], BF16, tag=f"kT{eb}", name=f"kT{eb}") for eb in range(2)]
            v_sb = [pp.tile([128, HL * 65], BF16, tag=f"v{st}", name=f"v{st}")
                    for st in range(ST)]

            # ---------- emitters ----------
            def emit_x_dma(st):
                x_t = xin_pool.tile([128, D], F32, tag="x_in", name="x_in",
                                    bufs=8)
                nc.sync.dma_start(x_t[:], x_ext.ap()[st * 128:(st + 1) * 128, :])
                return x_t

            def emit_x_tile(st, x_t):
                """cast bf16 (alternating ACT/DVE), PE-transpose into xT_q"""
                xb = xin_pool.tile([128, D], BF16, tag="x_bf", name="x_bf")
                if st % 2 == 0:
                    nc.scalar.copy(xb[:], x_t[:])
                else:
                    nc.vector.tensor_copy(xb[:], x_t[:])
                ps_t = ps_pool.tile([128, D], BF16, tag="psS", name="tp")
                for dc in range(DC):
                    nc.tensor.matmul(ps_t[:, dc * 128:(dc + 1) * 128],
                                     xb[:, dc * 128:(dc + 1) * 128], ident[:],
                                     is_transpose=True, start=(dc == 0),
                                     stop=(dc == DC - 1))
                q, r = st // 4, st % 4
                dst = xT_q[q][:].rearrange("p (d s) -> p d s", s=512)[:, :, r * 128:(r + 1) * 128]
                src_ap = ps_t[:].rearrange("p (d c) -> p d c", c=128)
                nc.vector.tensor_copy(dst, src_ap)

            def emit_proj(w1n, w2n, dst, eb, sb):
                """rope-folded projection of one [128, 512] block of qT/kT"""
                pj = psm_pool.tile([128, 1024], F32, tag="psM", name="pj")
                psA, psB = pj[:, 0:512], pj[:, 512:1024]
                for dc in range(DC):
                    nc.tensor.matmul(
                        psA, w_sb[w1n][dc][:, eb * 128:(eb + 1) * 128],
                        xT(dc, sb), start=(dc == 0), stop=(dc == DC - 1))
                for dc in range(DC):
                    nc.tensor.matmul(
                        psB, w_sb[w2n][dc][:, eb * 128:(eb + 1) * 128],
                        xT(dc, sb), start=(dc == 0), stop=(dc == DC - 1))
                t1 = wk_pool.tile([128, 512], F32, tag="ropeA", name="ropeA", bufs=6)
                nc.vector.tensor_tensor(
                    out=t1[:], in0=psA, in1=cos_sb[:, sb * 512:(sb + 1) * 512],
                    op=mybir.AluOpType.mult)
                t2 = wk_pool.tile([128, 512], F32, tag="ropeB", name="ropeB", bufs=6)
                nc.vector.tensor_tensor(
                    out=t2[:], in0=psB, in1=sin_sb[:, sb * 512:(sb + 1) * 512],
                    op=mybir.AluOpType.mult)
                nc.vector.tensor_tensor(
                    out=dst[eb][:, sb * 512:(sb + 1) * 512], in0=t1[:], in1=t2[:],
                    op=mybir.AluOpType.add)

            def emit_vproj(st):
                """v for one s-tile, [v_h | 1] layout per head (65 cols)"""
                psV = ps_pool.tile([128, EL], F32, tag="psS", name="pjV")
                for dc in range(DC):
                    nc.tensor.matmul(
                        psV[:], xT128(dc, st), w_sb["wv"][dc][:],
                        start=(dc == 0), stop=(dc == DC - 1))
                nc.vector.memset(v_sb[st][:], 1.0)
                out_ap = v_sb[st][:].rearrange("p (h x) -> p h x", x=65)[:, :, 0:64]
                in_ap = psV[:].rearrange("p (h x) -> p h x", x=64)
                nc.vector.tensor_copy(out_ap, in_ap)

            def emit_attn(sb, hp, weave=None):
                """scores+softmax+attn*v for head pair hp on s-block sb; the
                oT matmuls trail the score matmuls by LAG chunks so the
                in-order PE stream never head-blocks on exp latency"""
                eb = hp
                h0, h1 = 2 * hp, 2 * hp + 1
                ps_o2 = psm_pool.tile([128, 1024], F32, tag="psM", name="ps_o2")
                ps_o0, ps_o1 = ps_o2[0:65, 0:512], ps_o2[0:65, 512:1024]
                pts = {}

                def emit_scores(skc):
                    ps_s = ps_pool.tile([128, 1024], F32, tag="psS", name="ps_s")
                    nc.tensor.matmul(
                        ps_s[:, 0:512],
                        kT[eb][0:64, skc * 128:(skc + 1) * 128],
                        qT[eb][0:64, sb * 512:(sb + 1) * 512],
                        start=True, stop=True)
                    nc.tensor.matmul(
                        ps_s[:, 512:1024],
                        kT[eb][64:128, skc * 128:(skc + 1) * 128],
                        qT[eb][64:128, sb * 512:(sb + 1) * 512],
                        start=True, stop=True)
                    pt = wk_pool.tile([128, 1024], BF16, tag="p_exp", name="p_exp",
                                      bufs=LAG + 3)
                    nc.scalar.activation(pt[:], ps_s[:],
                                         mybir.ActivationFunctionType.Exp,
                                         scale=0.125)
                    pts[skc] = pt

                def emit_ot(skc):
                    pt = pts.pop(skc)
                    nc.tensor.matmul(ps_o0, v_sb[skc][:, h0 * 65:(h0 + 1) * 65],
                                     pt[:, 0:512],
                                     start=(skc == 0), stop=(skc == ST - 1))
                    nc.tensor.matmul(ps_o1, v_sb[skc][:, h1 * 65:(h1 + 1) * 65],
                                     pt[:, 512:1024],
                                     start=(skc == 0), stop=(skc == ST - 1))

                for skc in range(ST):
                    emit_scores(skc)
                    if weave is not None:
                        weave(skc)
                    if skc >= LAG:
                        emit_ot(skc - LAG)
                for skc in range(ST - LAG, ST):
                    emit_ot(skc)

                # normalize both heads with the two chains interleaved so the
                # DVE works during the gpsimd broadcast of the other head
                st_o = wk_pool.tile([128, 512], BF16, tag="oS", name="oS", bufs=4)
                lreps = []
                for hi, ps_o in ((0, ps_o0), (1, ps_o1)):
                    l_sb = wk_pool.tile([1, 512], F32, tag="l_sb", name="l_sb")
                    nc.vector.tensor_copy(l_sb[:], ps_o[64:65, :])
                    linv = wk_pool.tile([1, 512], F32, tag="linv", name="linv")
                    nc.vector.reciprocal_approx_fast(out=linv[:], in_=l_sb[:])
                    lrep = wk_pool.tile([64, 512], F32, tag="lrep", name="lrep")
                    nc.gpsimd.partition_broadcast(lrep[:], linv[:])
                    lreps.append(lrep)
                for hi, ps_o in ((0, ps_o0), (1, ps_o1)):
                    nc.vector.tensor_tensor(
                        out=st_o[hi * 64:(hi + 1) * 64, :], in0=ps_o[0:64, :],
                        in1=lreps[hi][:], op=mybir.AluOpType.mult)
                return st_o

            def emit_y(sb, oS):
                """output projection for s-block sb (normalization is folded
                into oS already)"""
                for stl in range(4):
                    row0 = sb * 512 + stl * 128
                    ps_y = psm_pool.tile([128, D], F32, tag="psM", name="ps_y")
                    for dh in range(2):
                        for hp in range(2):
                            nc.tensor.matmul(
                                ps_y[:, dh * 512:(dh + 1) * 512],
                                oS[hp][:, stl * 128:(stl + 1) * 128],
                                wo_sb[hp][:, dh * 512:(dh + 1) * 512],
                                start=(hp == 0), stop=(hp == 1))
                    y_sb = wk_pool.tile([128, D], F32, tag="y_sb", name="y_sb", bufs=2)
                    nc.vector.tensor_copy(y_sb[:], ps_y[:])
                    nc.sync.dma_start(out_ext.ap()[row0:row0 + 128, :], y_sb[:])

            # ---------- schedule ----------
            # x DMAs first (startup critical path), weights interleaved after
            x_tiles = {st: emit_x_dma(st) for st in range(6)}
            load_w("wk")
            load_w("wk2")
            x_tiles.update({st: emit_x_dma(st) for st in range(6, 8)})
            cos_sb = pp.tile([128, S], F32, tag="cos", name="cos")
            nc.sync.dma_start(cos_sb[:], cos_ext.ap())
            sin_sb = pp.tile([128, S], F32, tag="sin", name="sin")
            nc.sync.dma_start(sin_sb[:], sin_ext.ap())
            load_w("wv")
            for st in range(ST):
                if st + 8 < ST:
                    x_tiles[st + 8] = emit_x_dma(st + 8)
                emit_x_tile(st, x_tiles.pop(st))

            # remaining weights
            load_w("wq")
            load_w("wq2")
            wo_sb = []
            for hp in range(2):
                t = pp.tile([128, D], BF16, tag=f"wo{hp}", name=f"wo{hp}")
                nc.sync.dma_start(t[:], wo_ext.ap()[hp * 128:(hp + 1) * 128, :])
                wo_sb.append(t)

            for sbq in range(SBL):
                emit_proj("wk", "wk2", kT, 0, sbq)
            emit_proj("wq", "wq2", qT, 0, 0)

            # first attention block starts as early as possible; v-proj,
            # k(eb1) and q(eb1,0) are woven between its score chunks
            def weave0(skc):
                emit_vproj(skc)
                if skc % 4 == 2:
                    emit_proj("wk", "wk2", kT, 1, skc // 4)
                if skc == 14:
                    emit_proj("wq", "wq2", qT, 1, 0)

            pending = None
            for sb in range(SBL):
                oS = [emit_attn(sb, 0, weave=weave0 if sb == 0 else None)]
                if sb < SBL - 1:
                    emit_proj("wq", "wq2", qT, 0, sb + 1)
                if pending is not None:
                    emit_y(*pending)
                    pending = None
                oS.append(emit_attn(sb, 1))
                if sb < SBL - 1:
                    emit_proj("wq", "wq2", qT, 1, sb + 1)
                pending = (sb, oS)
            emit_y(*pending)

    nc.compile()
    return nc


def _host_inputs(x, wq, wk, wv, wo):
    """Build per-core input maps (host-side shard + RoPE table prep)."""
    x = np.asarray(x, dtype=np.float32)
    wq = np.asarray(wq, dtype=np.float32)
    wk = np.asarray(wk, dtype=np.float32)
    wv = np.asarray(wv, dtype=np.float32)
    wo = np.asarray(wo, dtype=np.float32)

    def swap_fold(w):  # [D, H, E] -> rope partner weights
        w2 = np.empty_like(w)
        w2[:, :, :E // 2] = -w[:, :, E // 2:]
        w2[:, :, E // 2:] = w[:, :, :E // 2]
        return w2

    wq2, wk2 = swap_fold(wq), swap_fold(wk)

    # rope tables, e-major [128, S]: partition p = (head%2)*64 + j
    pos = np.arange(S, dtype=np.float64)
    j = np.arange(E // 2, dtype=np.float64)
    timescale = MIN_TIMESCALE * (MAX_TIMESCALE / MIN_TIMESCALE) ** (2.0 * j / E)
    sinusoid = pos[None, :] / timescale[:, None]          # [32, S]
    cos32 = np.cos(sinusoid).astype(np.float32)
    sin32 = np.sin(sinusoid).astype(np.float32)
    cos_t = np.tile(cos32, (4, 1))                         # [128, S]
    sin_t = np.tile(sin32, (4, 1))
    ident = np.eye(128, dtype=ml_dtypes.bfloat16)

    bf = ml_dtypes.bfloat16
    in_maps = []
    for c in range(N_CORES):
        b, hg = c // 4, c % 4
        hsl = slice(hg * HL, (hg + 1) * HL)
        in_maps.append({
            "x": np.ascontiguousarray(x[b]),
            "wq": np.ascontiguousarray(wq[:, hsl].reshape(D, EL).astype(bf)),
            "wq2": np.ascontiguousarray(wq2[:, hsl].reshape(D, EL).astype(bf)),
            "wk": np.ascontiguousarray(wk[:, hsl].reshape(D, EL).astype(bf)),
            "wk2": np.ascontiguousarray(wk2[:, hsl].reshape(D, EL).astype(bf)),
            "wv": np.ascontiguousarray(wv[:, hsl].reshape(D, EL).astype(bf)),
            "wo": np.ascontiguousarray(
                wo[hg * EL:(hg + 1) * EL].astype(bf)),
            "cos_t": cos_t,
            "sin_t": sin_t,
            "ident": ident,
        })
    return in_maps


def kernel(x, wq, wk, wv, wo, _trace=False):
    if "nc" not in _CACHE:
        _CACHE["nc"] = _build_graph()
    nc = _CACHE["nc"]
    in_maps = _host_inputs(x, wq, wk, wv, wo)
    kw = {}
    if _trace:
        kw["trace"] = True
    res = run_bass_kernel_spmd(nc, in_maps, list(range(N_CORES)), **kw)
    _CACHE["last_exec_ns"] = res.exec_time_ns
    out = np.zeros((B, S, D), dtype=np.float32)
    for c in range(N_CORES):
        out[c // 4] += res.results[c]["out"]
    return out
